# revision 14
# baseline (speedup 1.0000x reference)
"""GQA attention (B=2,S=2048,D=2048,H=16,KV=4,HD=128) + RoPE on 8 TRN2 NeuronCores.

Sharding: core c -> (batch b=c//4, kv-group g=c%4). Each core projects
Q (4 heads), K/V (1 kv head) for its batch from a replicated x^T, applies
RoPE, runs causal flash attention (scores^T layout, no-max softmax --
|scores|<9 so fp32 exp is safe), AllGathers the per-head attention outputs
across the 4-core batch group, and computes a column slice of the output
projection (column-parallel wo).

Host-side prep (inside kernel()): transpose/cast inputs to bf16, expand
RoPE tables, build permutation/identity/mask constants. Host-side
post: transpose + concatenate the 8 output column-slices.
"""
import numpy as np
import ml_dtypes

import concourse.bass as bass
import concourse.mybir as mybir
import concourse.tile as tile
from concourse import bacc
from concourse.bass import ts
from concourse.bass_utils import run_bass_kernel_spmd

BF = mybir.dt.bfloat16
F32 = mybir.dt.float32
bf16 = ml_dtypes.bfloat16

B, S, D = 2, 2048, 2048
H, KV, HD = 16, 4, 128
NT = 4          # 512-token chunks
ND = 16         # 128-wide D chunks
NH = 4          # heads per core
SCALE = 1.0 / np.sqrt(HD)
RG = [[0, 1, 2, 3], [4, 5, 6, 7]]


def build_nc():
    nc = bacc.Bacc("TRN2", target_bir_lowering=False, debug=False, num_devices=8)
    xt_d = nc.dram_tensor("xt", [D, S], BF, kind="ExternalInput").ap()
    wqkv_d = nc.dram_tensor("wqkvT", [D, 768], BF, kind="ExternalInput").ap()
    woT_d = nc.dram_tensor("woT", [D, 512], BF, kind="ExternalInput").ap()
    cos_d = nc.dram_tensor("cose", [128, S], F32, kind="ExternalInput").ap()
    sin_d = nc.dram_tensor("sins", [128, S], F32, kind="ExternalInput").ap()
    mask_d = nc.dram_tensor("mask01", [128, 896], BF, kind="ExternalInput").ap()
    pswap_d = nc.dram_tensor("pswap", [128, 128], BF, kind="ExternalInput").ap()
    ident_d = nc.dram_tensor("ident", [128, 128], BF, kind="ExternalInput").ap()
    onesc_d = nc.dram_tensor("onesc", [128, 128], BF, kind="ExternalInput").ap()
    onesr_d = nc.dram_tensor("onesr", [1, 128], BF, kind="ExternalInput").ap()
    out_d = nc.dram_tensor("out", [512, S], F32, kind="ExternalOutput").ap()

    xt_r = xt_d.rearrange("(o p) t -> p o t", p=128)      # [128, 16, 2048]
    wqkv_r = wqkv_d.rearrange("(o p) m -> p o m", p=128)  # [128, 16, 768]
    woT_r = woT_d.rearrange("(o p) m -> p o m", p=128)    # [128, 16, 512]

    with tile.TileContext(nc) as tc:
        with (
            tc.tile_pool(name="consts", bufs=1) as consts,
            tc.tile_pool(name="io", bufs=2) as io,
            tc.tile_pool(name="work", bufs=3) as work,
            tc.tile_pool(name="psS", bufs=3, space="PSUM") as psS,
            tc.tile_pool(name="psA", bufs=3, space="PSUM") as psA,
            tc.tile_pool(name="psB", bufs=2, space="PSUM") as psB,
            tc.tile_pool(name="dram", bufs=1, space="DRAM") as dram,
        ):
            # ---- persistent SBUF; DMA emit order = availability order.
            # consts ride the gpsimd DGE queue so they overlap the xt/sync
            # queue instead of serializing in front of it.
            w_sb = consts.tile([128, ND, 768], BF, name="w_sb")
            for m in range(6):
                nc.gpsimd.dma_start(w_sb[:, :, ts(m, 128)], wqkv_r[:, :, ts(m, 128)])
            pswap_sb = consts.tile([128, 128], BF, name="pswap_sb")
            nc.gpsimd.dma_start(pswap_sb, pswap_d)
            cos_sb = consts.tile([128, S], F32, name="cos_sb")
            nc.gpsimd.dma_start(cos_sb, cos_d)
            sin_sb = consts.tile([128, S], F32, name="sin_sb")
            nc.gpsimd.dma_start(sin_sb, sin_d)
            ident_sb = consts.tile([128, 128], BF, name="ident_sb")
            nc.gpsimd.dma_start(ident_sb, ident_d)
            mask_sb = consts.tile([128, 896], BF, name="mask_sb")
            nc.gpsimd.dma_start(mask_sb, mask_d)
            onesc_sb = consts.tile([128, 128], BF, name="onesc_sb")
            nc.gpsimd.dma_start(onesc_sb, onesc_d)
            onesr_sb = consts.tile([1, 128], BF, name="onesr_sb")
            nc.gpsimd.dma_start(onesr_sb, onesr_d)

            qt_sb = consts.tile([128, NH, S], BF, name="qt_sb")   # Q^T, rope'd
            kt_sb = consts.tile([128, S], BF, name="kt_sb")       # K^T, rope'd
            v_sb = consts.tile([128, ND, HD], BF, name="v_sb")    # V [tok, hd] blocks

            ag_in = [[dram.tile([256, 512], BF, name=f"agin{i}_{p}")
                      for p in range(2)] for i in range(NT)]
            ag_out = [[dram.tile([1024, 512], BF, name=f"agout{i}_{p}")
                       for p in range(2)] for i in range(NT)]

            def proj_chunk(tc_i):
                xt_t = io.tile([128, ND, 512], BF, tag="io512", name="xt_t")
                qengs = ([nc.sync, nc.scalar, nc.scalar, nc.sync]
                         if tc_i == 0 else [nc.sync] * 4)
                for q in range(4):
                    qengs[q].dma_start(xt_t[:, 4 * q:4 * (q + 1), :],
                                       xt_r[:, 4 * q:4 * (q + 1), ts(tc_i, 512)])
                for m in range(6):  # 4 q heads, k, v
                    ps = psA.tile([128, 512], F32, tag="psA", name="ps_proj")
                    for d in range(ND):
                        nc.tensor.matmul(
                            ps, lhsT=w_sb[:, d, ts(m, 128)], rhs=xt_t[:, d, :],
                            start=(d == 0), stop=(d == ND - 1),
                        )
                    if m < 5:
                        # RoPE: out = raw*cos + swap(raw)*sin_signed
                        raw = work.tile([128, 512], BF, tag="rope_raw", name="raw")
                        nc.scalar.copy(raw, ps)
                        ps2 = psB.tile([128, 512], F32, tag="psB", name="ps_swap")
                        nc.tensor.matmul(ps2, lhsT=pswap_sb, rhs=raw,
                                         start=True, stop=True)
                        t1 = work.tile([128, 512], F32, tag="rope_t1", name="t1")
                        nc.vector.tensor_tensor(
                            t1, ps, cos_sb[:, ts(tc_i, 512)], mybir.AluOpType.mult)
                        t2 = work.tile([128, 512], F32, tag="rope_t2", name="t2")
                        nc.vector.tensor_tensor(
                            t2, ps2, sin_sb[:, ts(tc_i, 512)], mybir.AluOpType.mult)
                        dst = (qt_sb[:, m, ts(tc_i, 512)] if m < 4
                               else kt_sb[:, ts(tc_i, 512)])
                        nc.vector.tensor_tensor(dst, t1, t2, mybir.AluOpType.add)
                    else:
                        # V^T chunk -> bf16 -> transpose to [tok, hd] blocks
                        vraw = work.tile([128, 512], BF, tag="rope_raw", name="vraw")
                        nc.scalar.copy(vraw, ps)
                        for j in range(4):
                            pst = psB.tile([128, 128], BF, tag="psB", name="ps_vT")
                            nc.tensor.transpose(pst, vraw[:, ts(j, 128)], ident_sb)
                            nc.vector.tensor_copy(v_sb[:, 4 * tc_i + j, :], pst)

            def attn_chunk(qc):
                horder = [2, 3, 0, 1] if qc == NT - 1 else [0, 1, 2, 3]
                done = set()
                for h in horder:
                    ps_att = psB.tile([128, 512], F32, tag="psB", name="ps_att")
                    ps_den = psA.tile([128, 512], F32, tag="psA", name="ps_den")
                    nkb = 4 * qc + 4
                    for kb in range(nkb):
                        r = kb - 4 * qc
                        o = max(r, 0) * 128   # first q column this kb can see
                        ps_s = psS.tile([128, 512], F32, tag="psS", name="ps_s")
                        nc.tensor.matmul(
                            ps_s[:, o:], lhsT=kt_sb[:, ts(kb, 128)],
                            rhs=qt_sb[:, h, 512 * qc + o:512 * (qc + 1)],
                            start=True, stop=True)
                        pt = work.tile([128, 512], BF, tag="pt", name="pt")
                        nc.scalar.activation(
                            pt[:, o:], ps_s[:, o:],
                            mybir.ActivationFunctionType.Exp, scale=SCALE)
                        if r >= 0:  # causal 0/1 mask on the hull, post-exp
                            nc.vector.tensor_tensor(
                                pt[:, o:], pt[:, o:],
                                mask_sb[:, 384:896 - o],
                                mybir.AluOpType.mult)
                        nc.tensor.matmul(
                            ps_att[:, o:], lhsT=v_sb[:, kb, :], rhs=pt[:, o:],
                            start=(kb == 0), stop=(kb == nkb - 1))
                        nc.tensor.matmul(
                            ps_den[:, o:], lhsT=onesc_sb, rhs=pt[:, o:],
                            start=(kb == 0), stop=(kb == nkb - 1))
                    # ones[128,128] lhsT made ps_den the partition-broadcast den
                    bden = work.tile([128, 512], F32, tag="bden", name="bden")
                    nc.vector.reciprocal_approx_fast(bden, ps_den)
                    att = work.tile([128, 512], BF, tag="att", name="att")
                    nc.vector.tensor_tensor(att, ps_att, bden,
                                            mybir.AluOpType.mult)
                    nc.sync.dma_start(ag_in[qc][h // 2][ts(h % 2, 128), :], att)
                    done.add(h)
                    if (h | 1) in done and (h & ~1) in done:
                        nc.gpsimd.collective_compute(
                            "AllGather", mybir.AluOpType.bypass,
                            replica_groups=RG,
                            ins=[ag_in[qc][h // 2][:].opt()],
                            outs=[ag_out[qc][h // 2][:].opt()])

            def oproj_chunk(tc_i):
                rhs = io.tile([128, ND, 512], BF, tag="io512", name="oproj_rhs")
                nc.sync.dma_start(
                    rhs[:, :8, :],
                    ag_out[tc_i][0].rearrange("(o p) t -> p o t", p=128))
                nc.sync.dma_start(
                    rhs[:, 8:, :],
                    ag_out[tc_i][1].rearrange("(o p) t -> p o t", p=128))
                corder = (list(range(8, 16)) + list(range(8))
                          if tc_i == NT - 1 else list(range(ND)))
                for j in range(4):
                    ps_o = psA.tile([128, 512], F32, tag="psA", name="ps_o")
                    for ci, c in enumerate(corder):
                        nc.tensor.matmul(
                            ps_o, lhsT=woT_sb[:, c, ts(j, 128)], rhs=rhs[:, c, :],
                            start=(ci == 0), stop=(ci == ND - 1))
                    o32 = work.tile([128, 512], F32, tag="o32", name="o32")
                    nc.vector.tensor_copy(o32, ps_o)
                    nc.sync.dma_start(out_d[ts(j, 128), ts(tc_i, 512)], o32)

            for i in range(NT):
                proj_chunk(i)
                attn_chunk(i)
            woT_sb = consts.tile([128, ND, 512], BF, name="woT_sb")
            nc.gpsimd.dma_start(woT_sb, woT_r)
            for i in range(NT):
                oproj_chunk(i)

    nc.compile()
    return nc


def make_in_maps(x, freqs_cos, freqs_sin, wq, wk, wv, wo):
    fc = np.asarray(freqs_cos, np.float32)
    fs = np.asarray(freqs_sin, np.float32)
    cos_exp = np.ascontiguousarray(np.repeat(fc.T, 2, axis=0))      # [128, S]
    sgn = np.tile(np.array([-1.0, 1.0], np.float32), 64)[:, None]
    sin_sgn = np.ascontiguousarray(np.repeat(fs.T, 2, axis=0) * sgn)
    mask01 = np.triu(np.ones((128, 896), np.float32), 384).astype(bf16)
    pswap = np.zeros((128, 128), np.float32)
    pswap[np.arange(128), np.arange(128) ^ 1] = 1.0
    pswap = pswap.astype(bf16)
    ident = np.eye(128, dtype=np.float32).astype(bf16)
    onesc = np.ones((128, 128), np.float32).astype(bf16)
    onesr = np.ones((1, 128), np.float32).astype(bf16)

    xt = [np.ascontiguousarray(np.asarray(x[b], np.float32).T).astype(bf16)
          for b in range(B)]
    in_maps = []
    for core in range(8):
        b, g = divmod(core, 4)
        wqkvT = np.concatenate(
            [np.asarray(wq, np.float32)[512 * g:512 * (g + 1)].T,
             np.asarray(wk, np.float32)[128 * g:128 * (g + 1)].T,
             np.asarray(wv, np.float32)[128 * g:128 * (g + 1)].T], axis=1)
        order = [0, 1, 4, 5, 8, 9, 12, 13, 2, 3, 6, 7, 10, 11, 14, 15]
        woT = np.asarray(wo, np.float32)[512 * g:512 * (g + 1), :].T
        woT = woT.reshape(16, 128, 512)[order].reshape(2048, 512)
        in_maps.append({
            "xt": xt[b],
            "wqkvT": np.ascontiguousarray(wqkvT).astype(bf16),
            "woT": np.ascontiguousarray(woT).astype(bf16),
            "cose": cos_exp,
            "sins": sin_sgn,
            "mask01": mask01,
            "pswap": pswap,
            "ident": ident,
            "onesc": onesc,
            "onesr": onesr,
        })
    return in_maps


_NC = None


def get_nc():
    global _NC
    if _NC is None:
        _NC = build_nc()
    return _NC


def assemble_out(results):
    out = np.zeros((B, S, D), np.float32)
    for core in range(8):
        b, g = divmod(core, 4)
        out[b, :, 512 * g:512 * (g + 1)] = results[core]["out"].T
    return out


def kernel(x, freqs_cos, freqs_sin, wq, wk, wv, wo):
    nc = get_nc()
    in_maps = make_in_maps(x, freqs_cos, freqs_sin, wq, wk, wv, wo)
    res = run_bass_kernel_spmd(nc, in_maps, core_ids=list(range(8)))
    return assemble_out(res.results)


# revision 15
# speedup vs baseline: 1.0493x; 1.0493x over previous
"""GQA attention (B=2,S=2048,D=2048,H=16,KV=4,HD=128) + RoPE on 8 TRN2 NeuronCores.

Sharding: core c -> (batch b=c//4, kv-group g=c%4). Each core projects
Q (4 heads), K/V (1 kv head) for its batch from a replicated x^T, applies
RoPE, runs causal flash attention (scores^T layout, no-max softmax --
|scores|<9 so fp32 exp is safe), AllGathers the per-head attention outputs
across the 4-core batch group, and computes a column slice of the output
projection (column-parallel wo).

Host-side prep (inside kernel()): transpose/cast inputs to bf16, expand
RoPE tables, build permutation/identity/mask constants. Host-side
post: transpose + concatenate the 8 output column-slices.
"""
import numpy as np
import ml_dtypes

import concourse.bass as bass
import concourse.mybir as mybir
import concourse.tile as tile
from concourse import bacc
from concourse.bass import ts
from concourse.bass_utils import run_bass_kernel_spmd

BF = mybir.dt.bfloat16
F32 = mybir.dt.float32
bf16 = ml_dtypes.bfloat16

B, S, D = 2, 2048, 2048
H, KV, HD = 16, 4, 128
NT = 4          # 512-token chunks
ND = 16         # 128-wide D chunks
NH = 4          # heads per core
SCALE = 1.0 / np.sqrt(HD)
RG = [[0, 1, 2, 3], [4, 5, 6, 7]]


def build_nc():
    nc = bacc.Bacc("TRN2", target_bir_lowering=False, debug=False, num_devices=8)
    xt_d = nc.dram_tensor("xt", [D, S], BF, kind="ExternalInput").ap()
    wqkv_d = nc.dram_tensor("wqkvT", [D, 768], BF, kind="ExternalInput").ap()
    woT_d = nc.dram_tensor("woT", [D, 512], BF, kind="ExternalInput").ap()
    cos_d = nc.dram_tensor("cose", [128, S], F32, kind="ExternalInput").ap()
    sin_d = nc.dram_tensor("sins", [128, S], F32, kind="ExternalInput").ap()
    mask_d = nc.dram_tensor("mask01", [128, 896], BF, kind="ExternalInput").ap()
    pswap_d = nc.dram_tensor("pswap", [128, 128], BF, kind="ExternalInput").ap()
    ident_d = nc.dram_tensor("ident", [128, 128], BF, kind="ExternalInput").ap()
    onesc_d = nc.dram_tensor("onesc", [128, 128], BF, kind="ExternalInput").ap()
    onesr_d = nc.dram_tensor("onesr", [1, 128], BF, kind="ExternalInput").ap()
    out_d = nc.dram_tensor("out", [512, S], F32, kind="ExternalOutput").ap()

    xt_r = xt_d.rearrange("(o p) t -> p o t", p=128)      # [128, 16, 2048]
    wqkv_r = wqkv_d.rearrange("(o p) m -> p o m", p=128)  # [128, 16, 768]
    woT_r = woT_d.rearrange("(o p) m -> p o m", p=128)    # [128, 16, 512]

    with tile.TileContext(nc) as tc:
        with (
            tc.tile_pool(name="consts", bufs=1) as consts,
            tc.tile_pool(name="io", bufs=2) as io,
            tc.tile_pool(name="work", bufs=3) as work,
            tc.tile_pool(name="psS", bufs=3, space="PSUM") as psS,
            tc.tile_pool(name="psA", bufs=3, space="PSUM") as psA,
            tc.tile_pool(name="psB", bufs=2, space="PSUM") as psB,
            tc.tile_pool(name="dram", bufs=1, space="DRAM") as dram,
        ):
            # ---- persistent SBUF; DMA emit order = availability order.
            # consts ride the gpsimd DGE queue so they overlap the xt/sync
            # queue instead of serializing in front of it.
            w_sb = consts.tile([128, ND, 768], BF, name="w_sb")
            for m in range(6):
                nc.gpsimd.dma_start(w_sb[:, :, ts(m, 128)], wqkv_r[:, :, ts(m, 128)])
            pswap_sb = consts.tile([128, 128], BF, name="pswap_sb")
            nc.gpsimd.dma_start(pswap_sb, pswap_d)
            cos_sb = consts.tile([128, S], F32, name="cos_sb")
            nc.gpsimd.dma_start(cos_sb, cos_d)
            sin_sb = consts.tile([128, S], F32, name="sin_sb")
            nc.gpsimd.dma_start(sin_sb, sin_d)
            ident_sb = consts.tile([128, 128], BF, name="ident_sb")
            nc.gpsimd.dma_start(ident_sb, ident_d)
            mask_sb = consts.tile([128, 896], BF, name="mask_sb")
            nc.gpsimd.dma_start(mask_sb, mask_d)
            onesc_sb = consts.tile([128, 128], BF, name="onesc_sb")
            nc.gpsimd.dma_start(onesc_sb, onesc_d)
            onesr_sb = consts.tile([1, 128], BF, name="onesr_sb")
            nc.gpsimd.dma_start(onesr_sb, onesr_d)

            qt_sb = consts.tile([128, NH, S], BF, name="qt_sb")   # Q^T, rope'd
            kt_sb = consts.tile([128, S], BF, name="kt_sb")       # K^T, rope'd
            v_sb = consts.tile([128, ND, HD], BF, name="v_sb")    # V [tok, hd] blocks

            ag_in = [[dram.tile([256, 512], BF, name=f"agin{i}_{p}")
                      for p in range(2)] for i in range(NT)]
            ag_out = [[dram.tile([1024, 512], BF, name=f"agout{i}_{p}")
                       for p in range(2)] for i in range(NT)]

            def proj_chunk(tc_i):
                xt_t = io.tile([128, ND, 512], BF, tag="io512", name="xt_t")
                qengs = ([nc.sync, nc.scalar, nc.scalar, nc.sync]
                         if tc_i == 0 else [nc.sync] * 4)
                for q in range(4):
                    qengs[q].dma_start(xt_t[:, 4 * q:4 * (q + 1), :],
                                       xt_r[:, 4 * q:4 * (q + 1), ts(tc_i, 512)])
                for m in range(6):  # 4 q heads, k, v
                    ps = psA.tile([128, 512], F32, tag="psA", name="ps_proj")
                    for d in range(ND):
                        nc.tensor.matmul(
                            ps, lhsT=w_sb[:, d, ts(m, 128)], rhs=xt_t[:, d, :],
                            start=(d == 0), stop=(d == ND - 1),
                        )
                    if m < 5:
                        # RoPE: out = raw*cos + swap(raw)*sin_signed
                        raw = work.tile([128, 512], BF, tag="rope_raw", name="raw")
                        nc.scalar.copy(raw, ps)
                        ps2 = psB.tile([128, 512], F32, tag="psB", name="ps_swap")
                        nc.tensor.matmul(ps2, lhsT=pswap_sb, rhs=raw,
                                         start=True, stop=True)
                        t1 = work.tile([128, 512], F32, tag="rope_t1", name="t1")
                        nc.vector.tensor_tensor(
                            t1, ps, cos_sb[:, ts(tc_i, 512)], mybir.AluOpType.mult)
                        t2 = work.tile([128, 512], F32, tag="rope_t2", name="t2")
                        nc.vector.tensor_tensor(
                            t2, ps2, sin_sb[:, ts(tc_i, 512)], mybir.AluOpType.mult)
                        dst = (qt_sb[:, m, ts(tc_i, 512)] if m < 4
                               else kt_sb[:, ts(tc_i, 512)])
                        nc.vector.tensor_tensor(dst, t1, t2, mybir.AluOpType.add)
                    else:
                        # V^T chunk -> bf16 -> transpose to [tok, hd] blocks
                        vraw = work.tile([128, 512], BF, tag="rope_raw", name="vraw")
                        nc.scalar.copy(vraw, ps)
                        for j in range(4):
                            pst = psB.tile([128, 128], BF, tag="psB", name="ps_vT")
                            nc.tensor.transpose(pst, vraw[:, ts(j, 128)], ident_sb)
                            nc.vector.tensor_copy(v_sb[:, 4 * tc_i + j, :], pst)

            def attn_chunk(qc):
                for h in range(NH):
                    ps_att = psB.tile([128, 512], F32, tag="psB", name="ps_att")
                    ps_den = psA.tile([128, 512], F32, tag="psA", name="ps_den")
                    nkb = 4 * qc + 4
                    for kb in range(nkb):
                        r = kb - 4 * qc
                        o = max(r, 0) * 128   # first q column this kb can see
                        ps_s = psS.tile([128, 512], F32, tag="psS", name="ps_s")
                        nc.tensor.matmul(
                            ps_s[:, o:], lhsT=kt_sb[:, ts(kb, 128)],
                            rhs=qt_sb[:, h, 512 * qc + o:512 * (qc + 1)],
                            start=True, stop=True)
                        pt = work.tile([128, 512], BF, tag="pt", name="pt")
                        nc.scalar.activation(
                            pt[:, o:], ps_s[:, o:],
                            mybir.ActivationFunctionType.Exp, scale=SCALE)
                        if r >= 0:  # causal 0/1 mask on the hull, post-exp
                            nc.vector.tensor_tensor(
                                pt[:, o:], pt[:, o:],
                                mask_sb[:, 384:896 - o],
                                mybir.AluOpType.mult)
                        nc.tensor.matmul(
                            ps_att[:, o:], lhsT=v_sb[:, kb, :], rhs=pt[:, o:],
                            start=(kb == 0), stop=(kb == nkb - 1))
                        nc.tensor.matmul(
                            ps_den[:, o:], lhsT=onesc_sb, rhs=pt[:, o:],
                            start=(kb == 0), stop=(kb == nkb - 1))
                    # ones[128,128] lhsT made ps_den the partition-broadcast den
                    bden = work.tile([128, 512], F32, tag="bden", name="bden")
                    nc.vector.reciprocal_approx_fast(bden, ps_den)
                    att = work.tile([128, 512], BF, tag="att", name="att")
                    nc.vector.tensor_tensor(att, ps_att, bden,
                                            mybir.AluOpType.mult)
                    nc.scalar.dma_start(ag_in[qc][h // 2][ts(h % 2, 128), :], att)
                    if h % 2 == 1:
                        nc.gpsimd.collective_compute(
                            "AllGather", mybir.AluOpType.bypass,
                            replica_groups=RG,
                            ins=[ag_in[qc][h // 2][:].opt()],
                            outs=[ag_out[qc][h // 2][:].opt()])

            def oproj_chunk(tc_i):
                rhs = io.tile([128, ND, 512], BF, tag="io512", name="oproj_rhs")
                nc.sync.dma_start(
                    rhs[:, :8, :],
                    ag_out[tc_i][0].rearrange("(o p) t -> p o t", p=128))
                nc.sync.dma_start(
                    rhs[:, 8:, :],
                    ag_out[tc_i][1].rearrange("(o p) t -> p o t", p=128))
                for j in range(4):
                    ps_o = psA.tile([128, 512], F32, tag="psA", name="ps_o")
                    for c in range(ND):
                        nc.tensor.matmul(
                            ps_o, lhsT=woT_sb[:, c, ts(j, 128)], rhs=rhs[:, c, :],
                            start=(c == 0), stop=(c == ND - 1))
                    o32 = work.tile([128, 512], F32, tag="o32", name="o32")
                    nc.vector.tensor_copy(o32, ps_o)
                    nc.sync.dma_start(out_d[ts(j, 128), ts(tc_i, 512)], o32)

            for i in range(NT):
                proj_chunk(i)
                attn_chunk(i)
            woT_sb = consts.tile([128, ND, 512], BF, name="woT_sb")
            nc.gpsimd.dma_start(woT_sb, woT_r)
            for i in range(NT):
                oproj_chunk(i)

    nc.compile()
    return nc


def make_in_maps(x, freqs_cos, freqs_sin, wq, wk, wv, wo):
    fc = np.asarray(freqs_cos, np.float32)
    fs = np.asarray(freqs_sin, np.float32)
    cos_exp = np.ascontiguousarray(np.repeat(fc.T, 2, axis=0))      # [128, S]
    sgn = np.tile(np.array([-1.0, 1.0], np.float32), 64)[:, None]
    sin_sgn = np.ascontiguousarray(np.repeat(fs.T, 2, axis=0) * sgn)
    mask01 = np.triu(np.ones((128, 896), np.float32), 384).astype(bf16)
    pswap = np.zeros((128, 128), np.float32)
    pswap[np.arange(128), np.arange(128) ^ 1] = 1.0
    pswap = pswap.astype(bf16)
    ident = np.eye(128, dtype=np.float32).astype(bf16)
    onesc = np.ones((128, 128), np.float32).astype(bf16)
    onesr = np.ones((1, 128), np.float32).astype(bf16)

    xt = [np.ascontiguousarray(np.asarray(x[b], np.float32).T).astype(bf16)
          for b in range(B)]
    in_maps = []
    for core in range(8):
        b, g = divmod(core, 4)
        wqkvT = np.concatenate(
            [np.asarray(wq, np.float32)[512 * g:512 * (g + 1)].T,
             np.asarray(wk, np.float32)[128 * g:128 * (g + 1)].T,
             np.asarray(wv, np.float32)[128 * g:128 * (g + 1)].T], axis=1)
        order = [0, 1, 4, 5, 8, 9, 12, 13, 2, 3, 6, 7, 10, 11, 14, 15]
        woT = np.asarray(wo, np.float32)[512 * g:512 * (g + 1), :].T
        woT = woT.reshape(16, 128, 512)[order].reshape(2048, 512)
        in_maps.append({
            "xt": xt[b],
            "wqkvT": np.ascontiguousarray(wqkvT).astype(bf16),
            "woT": np.ascontiguousarray(woT).astype(bf16),
            "cose": cos_exp,
            "sins": sin_sgn,
            "mask01": mask01,
            "pswap": pswap,
            "ident": ident,
            "onesc": onesc,
            "onesr": onesr,
        })
    return in_maps


_NC = None


def get_nc():
    global _NC
    if _NC is None:
        _NC = build_nc()
    return _NC


def assemble_out(results):
    out = np.zeros((B, S, D), np.float32)
    for core in range(8):
        b, g = divmod(core, 4)
        out[b, :, 512 * g:512 * (g + 1)] = results[core]["out"].T
    return out


def kernel(x, freqs_cos, freqs_sin, wq, wk, wv, wo):
    nc = get_nc()
    in_maps = make_in_maps(x, freqs_cos, freqs_sin, wq, wk, wv, wo)
    res = run_bass_kernel_spmd(nc, in_maps, core_ids=list(range(8)))
    return assemble_out(res.results)


# revision 16
# speedup vs baseline: 1.0575x; 1.0078x over previous
"""GQA attention (B=2,S=2048,D=2048,H=16,KV=4,HD=128) + RoPE on 8 TRN2 NeuronCores.

Sharding: core c -> (batch b=c//4, kv-group g=c%4). Each core projects
Q (4 heads), K/V (1 kv head) for its batch from a replicated x^T, applies
RoPE, runs causal flash attention (scores^T layout, no-max softmax --
|scores|<9 so fp32 exp is safe), AllGathers the per-head attention outputs
across the 4-core batch group, and computes a column slice of the output
projection (column-parallel wo).

Host-side prep (inside kernel()): transpose/cast inputs to bf16, expand
RoPE tables, build permutation/identity/mask constants. Host-side
post: transpose + concatenate the 8 output column-slices.
"""
import numpy as np
import ml_dtypes

import concourse.bass as bass
import concourse.mybir as mybir
import concourse.tile as tile
from concourse import bacc
from concourse.bass import ts
from concourse.bass_utils import run_bass_kernel_spmd

BF = mybir.dt.bfloat16
F32 = mybir.dt.float32
bf16 = ml_dtypes.bfloat16

B, S, D = 2, 2048, 2048
H, KV, HD = 16, 4, 128
NT = 4          # 512-token chunks
ND = 16         # 128-wide D chunks
NH = 4          # heads per core
SCALE = 1.0 / np.sqrt(HD)
RG = [[0, 1, 2, 3], [4, 5, 6, 7]]


def build_nc():
    nc = bacc.Bacc("TRN2", target_bir_lowering=False, debug=False, num_devices=8)
    xt_d = nc.dram_tensor("xt", [D, S], BF, kind="ExternalInput").ap()
    wqkv_d = nc.dram_tensor("wqkvT", [6, 128, 2048], BF, kind="ExternalInput").ap()
    woT_d = nc.dram_tensor("woT", [D, 512], BF, kind="ExternalInput").ap()
    cos_d = nc.dram_tensor("cose", [128, S], F32, kind="ExternalInput").ap()
    sin_d = nc.dram_tensor("sins", [128, S], F32, kind="ExternalInput").ap()
    mask_d = nc.dram_tensor("mask01", [128, 896], BF, kind="ExternalInput").ap()
    pswap_d = nc.dram_tensor("pswap", [128, 128], BF, kind="ExternalInput").ap()
    ident_d = nc.dram_tensor("ident", [128, 128], BF, kind="ExternalInput").ap()
    onesc_d = nc.dram_tensor("onesc", [128, 128], BF, kind="ExternalInput").ap()
    onesr_d = nc.dram_tensor("onesr", [1, 128], BF, kind="ExternalInput").ap()
    out_d = nc.dram_tensor("out", [512, S], F32, kind="ExternalOutput").ap()

    xt_r = xt_d.rearrange("(o p) t -> p o t", p=128)      # [128, 16, 2048]
    woT_r = woT_d.rearrange("(o p) m -> p o m", p=128)    # [128, 16, 512]

    with tile.TileContext(nc) as tc:
        with (
            tc.tile_pool(name="consts", bufs=1) as consts,
            tc.tile_pool(name="io", bufs=2) as io,
            tc.tile_pool(name="work", bufs=3) as work,
            tc.tile_pool(name="psS", bufs=3, space="PSUM") as psS,
            tc.tile_pool(name="psA", bufs=3, space="PSUM") as psA,
            tc.tile_pool(name="psB", bufs=2, space="PSUM") as psB,
            tc.tile_pool(name="dram", bufs=1, space="DRAM") as dram,
        ):
            # ---- persistent SBUF; DMA emit order = availability order.
            # consts ride the gpsimd DGE queue so they overlap the xt/sync
            # queue instead of serializing in front of it.
            w_sb = consts.tile([128, 6, ND, 128], BF, name="w_sb")
            for m in range(6):
                nc.gpsimd.dma_start(
                    w_sb[:, m], wqkv_d[m].rearrange("p (o c) -> p o c", c=128))
            pswap_sb = consts.tile([128, 128], BF, name="pswap_sb")
            nc.gpsimd.dma_start(pswap_sb, pswap_d)
            cos_sb = consts.tile([128, S], F32, name="cos_sb")
            nc.gpsimd.dma_start(cos_sb, cos_d)
            sin_sb = consts.tile([128, S], F32, name="sin_sb")
            nc.gpsimd.dma_start(sin_sb, sin_d)
            ident_sb = consts.tile([128, 128], BF, name="ident_sb")
            nc.gpsimd.dma_start(ident_sb, ident_d)
            mask_sb = consts.tile([128, 896], BF, name="mask_sb")
            nc.gpsimd.dma_start(mask_sb, mask_d)
            onesc_sb = consts.tile([128, 128], BF, name="onesc_sb")
            nc.gpsimd.dma_start(onesc_sb, onesc_d)
            onesr_sb = consts.tile([1, 128], BF, name="onesr_sb")
            nc.gpsimd.dma_start(onesr_sb, onesr_d)

            qt_sb = consts.tile([128, NH, S], BF, name="qt_sb")   # Q^T, rope'd
            kt_sb = consts.tile([128, S], BF, name="kt_sb")       # K^T, rope'd
            v_sb = consts.tile([128, ND, HD], BF, name="v_sb")    # V [tok, hd] blocks

            ag_in = [[dram.tile([256, 512], BF, name=f"agin{i}_{p}")
                      for p in range(2)] for i in range(NT)]
            ag_out = [[dram.tile([1024, 512], BF, name=f"agout{i}_{p}")
                       for p in range(2)] for i in range(NT)]

            def proj_chunk(tc_i):
                xt_t = io.tile([128, ND, 512], BF, tag="io512", name="xt_t")
                qengs = ([nc.sync, nc.scalar, nc.scalar, nc.sync]
                         if tc_i == 0 else [nc.sync] * 4)
                for q in range(4):
                    qengs[q].dma_start(xt_t[:, 4 * q:4 * (q + 1), :],
                                       xt_r[:, 4 * q:4 * (q + 1), ts(tc_i, 512)])
                for m in range(6):  # 4 q heads, k, v
                    ps = psA.tile([128, 512], F32, tag="psA", name="ps_proj")
                    for d in range(ND):
                        nc.tensor.matmul(
                            ps, lhsT=w_sb[:, m, d, :], rhs=xt_t[:, d, :],
                            start=(d == 0), stop=(d == ND - 1),
                        )
                    if m < 5:
                        # RoPE: out = raw*cos + swap(raw)*sin_signed
                        raw = work.tile([128, 512], BF, tag="rope_raw", name="raw")
                        nc.scalar.copy(raw, ps)
                        ps2 = psB.tile([128, 512], F32, tag="psB", name="ps_swap")
                        nc.tensor.matmul(ps2, lhsT=pswap_sb, rhs=raw,
                                         start=True, stop=True)
                        t1 = work.tile([128, 512], F32, tag="rope_t1", name="t1")
                        nc.vector.tensor_tensor(
                            t1, ps, cos_sb[:, ts(tc_i, 512)], mybir.AluOpType.mult)
                        t2 = work.tile([128, 512], F32, tag="rope_t2", name="t2")
                        nc.vector.tensor_tensor(
                            t2, ps2, sin_sb[:, ts(tc_i, 512)], mybir.AluOpType.mult)
                        dst = (qt_sb[:, m, ts(tc_i, 512)] if m < 4
                               else kt_sb[:, ts(tc_i, 512)])
                        nc.vector.tensor_tensor(dst, t1, t2, mybir.AluOpType.add)
                    else:
                        # V^T chunk -> bf16 -> transpose to [tok, hd] blocks
                        vraw = work.tile([128, 512], BF, tag="rope_raw", name="vraw")
                        nc.scalar.copy(vraw, ps)
                        for j in range(4):
                            pst = psB.tile([128, 128], BF, tag="psB", name="ps_vT")
                            nc.tensor.transpose(pst, vraw[:, ts(j, 128)], ident_sb)
                            nc.vector.tensor_copy(v_sb[:, 4 * tc_i + j, :], pst)

            def attn_chunk(qc):
                for h in range(NH):
                    ps_att = psB.tile([128, 512], F32, tag="psB", name="ps_att")
                    ps_den = psA.tile([128, 512], F32, tag="psA", name="ps_den")
                    nkb = 4 * qc + 4
                    for kb in range(nkb):
                        r = kb - 4 * qc
                        o = max(r, 0) * 128   # first q column this kb can see
                        ps_s = psS.tile([128, 512], F32, tag="psS", name="ps_s")
                        nc.tensor.matmul(
                            ps_s[:, o:], lhsT=kt_sb[:, ts(kb, 128)],
                            rhs=qt_sb[:, h, 512 * qc + o:512 * (qc + 1)],
                            start=True, stop=True)
                        pt = work.tile([128, 512], BF, tag="pt", name="pt")
                        nc.scalar.activation(
                            pt[:, o:], ps_s[:, o:],
                            mybir.ActivationFunctionType.Exp, scale=SCALE)
                        if r >= 0:  # causal 0/1 mask on the hull, post-exp
                            nc.vector.tensor_tensor(
                                pt[:, o:], pt[:, o:],
                                mask_sb[:, 384:896 - o],
                                mybir.AluOpType.mult)
                        nc.tensor.matmul(
                            ps_att[:, o:], lhsT=v_sb[:, kb, :], rhs=pt[:, o:],
                            start=(kb == 0), stop=(kb == nkb - 1))
                        nc.tensor.matmul(
                            ps_den[:, o:], lhsT=onesc_sb, rhs=pt[:, o:],
                            start=(kb == 0), stop=(kb == nkb - 1))
                    # ones[128,128] lhsT made ps_den the partition-broadcast den
                    bden = work.tile([128, 512], F32, tag="bden", name="bden")
                    nc.vector.reciprocal_approx_fast(bden, ps_den)
                    att = work.tile([128, 512], BF, tag="att", name="att")
                    nc.vector.tensor_tensor(att, ps_att, bden,
                                            mybir.AluOpType.mult)
                    nc.scalar.dma_start(ag_in[qc][h // 2][ts(h % 2, 128), :], att)
                    if h % 2 == 1:
                        nc.gpsimd.collective_compute(
                            "AllGather", mybir.AluOpType.bypass,
                            replica_groups=RG,
                            ins=[ag_in[qc][h // 2][:].opt()],
                            outs=[ag_out[qc][h // 2][:].opt()])

            def oproj_chunk(tc_i):
                rhs = io.tile([128, ND, 512], BF, tag="io512", name="oproj_rhs")
                nc.sync.dma_start(
                    rhs[:, :8, :],
                    ag_out[tc_i][0].rearrange("(o p) t -> p o t", p=128))
                nc.sync.dma_start(
                    rhs[:, 8:, :],
                    ag_out[tc_i][1].rearrange("(o p) t -> p o t", p=128))
                for j in range(4):
                    ps_o = psA.tile([128, 512], F32, tag="psA", name="ps_o")
                    for c in range(ND):
                        nc.tensor.matmul(
                            ps_o, lhsT=woT_sb[:, c, ts(j, 128)], rhs=rhs[:, c, :],
                            start=(c == 0), stop=(c == ND - 1))
                    o32 = work.tile([128, 512], F32, tag="o32", name="o32")
                    nc.vector.tensor_copy(o32, ps_o)
                    nc.sync.dma_start(out_d[ts(j, 128), ts(tc_i, 512)], o32)

            for i in range(NT):
                proj_chunk(i)
                attn_chunk(i)
            woT_sb = consts.tile([128, ND, 512], BF, name="woT_sb")
            nc.gpsimd.dma_start(woT_sb, woT_r)
            for i in range(NT):
                oproj_chunk(i)

    nc.compile()
    return nc


def make_in_maps(x, freqs_cos, freqs_sin, wq, wk, wv, wo):
    fc = np.asarray(freqs_cos, np.float32)
    fs = np.asarray(freqs_sin, np.float32)
    cos_exp = np.ascontiguousarray(np.repeat(fc.T, 2, axis=0))      # [128, S]
    sgn = np.tile(np.array([-1.0, 1.0], np.float32), 64)[:, None]
    sin_sgn = np.ascontiguousarray(np.repeat(fs.T, 2, axis=0) * sgn)
    mask01 = np.triu(np.ones((128, 896), np.float32), 384).astype(bf16)
    pswap = np.zeros((128, 128), np.float32)
    pswap[np.arange(128), np.arange(128) ^ 1] = 1.0
    pswap = pswap.astype(bf16)
    ident = np.eye(128, dtype=np.float32).astype(bf16)
    onesc = np.ones((128, 128), np.float32).astype(bf16)
    onesr = np.ones((1, 128), np.float32).astype(bf16)

    xt = [np.ascontiguousarray(np.asarray(x[b], np.float32).T).astype(bf16)
          for b in range(B)]
    in_maps = []
    for core in range(8):
        b, g = divmod(core, 4)
        wqkvT = np.concatenate(
            [np.asarray(wq, np.float32)[512 * g:512 * (g + 1)].T,
             np.asarray(wk, np.float32)[128 * g:128 * (g + 1)].T,
             np.asarray(wv, np.float32)[128 * g:128 * (g + 1)].T], axis=1)
        # m-major SBUF-order blocks: [6][p 128][o*128+c 2048]
        wqkvT = np.ascontiguousarray(
            wqkvT.reshape(16, 128, 768).transpose(2, 1, 0)   # [768 m, 128 p, 16 o]
        )  # temp
        wqkvT = np.ascontiguousarray(np.stack(
            [wqkvT[128 * m:128 * (m + 1)].transpose(1, 2, 0).reshape(128, 2048)
             for m in range(6)]))
        order = [0, 1, 4, 5, 8, 9, 12, 13, 2, 3, 6, 7, 10, 11, 14, 15]
        woT = np.asarray(wo, np.float32)[512 * g:512 * (g + 1), :].T
        woT = woT.reshape(16, 128, 512)[order].reshape(2048, 512)
        in_maps.append({
            "xt": xt[b],
            "wqkvT": np.ascontiguousarray(wqkvT).astype(bf16),
            "woT": np.ascontiguousarray(woT).astype(bf16),
            "cose": cos_exp,
            "sins": sin_sgn,
            "mask01": mask01,
            "pswap": pswap,
            "ident": ident,
            "onesc": onesc,
            "onesr": onesr,
        })
    return in_maps


_NC = None


def get_nc():
    global _NC
    if _NC is None:
        _NC = build_nc()
    return _NC


def assemble_out(results):
    out = np.zeros((B, S, D), np.float32)
    for core in range(8):
        b, g = divmod(core, 4)
        out[b, :, 512 * g:512 * (g + 1)] = results[core]["out"].T
    return out


def kernel(x, freqs_cos, freqs_sin, wq, wk, wv, wo):
    nc = get_nc()
    in_maps = make_in_maps(x, freqs_cos, freqs_sin, wq, wk, wv, wo)
    res = run_bass_kernel_spmd(nc, in_maps, core_ids=list(range(8)))
    return assemble_out(res.results)
